# revision 18
# baseline (speedup 1.0000x reference)
"""ConvLSTM block Trainium2 kernel (8 NeuronCores).

Sharding: 8 cores = 4 batches x 2 H-halves. Bottom-half cores process their
slab vertically flipped (with kh-flipped conv kernels) so one SPMD program
serves all cores.

Halo scheme: instead of exchanging one boundary row of h every step (15
AllReduces of ~8-16us latency each on the critical path), the cores exchange
a 4-row halo of (h, c) every 4 steps (after steps 3, 7, 11 -> 3 AllReduces).
Between exchanges each core redundantly computes a shrinking wedge of the
neighbor's rows (3, 2, 1, 0 extra rows per step within a phase) in a small
"mini" PSUM group, keeping all owned rows exact at every step.

Per-core compute per step: for each of 2 output-channel tiles (128 ch) and
each chunk of 8 output rows (N=512 pixels), one PSUM accumulation group of
9 matmuls: 3x input conv (K=96: 3 row-taps x 32ch; stride-2 column access
via strided APs) + 6x recurrent conv (K=128: 2 row-taps x 64ch using a
row-shifted duplicate copy of h in partitions 64:128). Gates/LSTM/BN run on
ACT+DVE out of PSUM; bias and BN are folded into the activations.

Scheduling: on exchange steps the boundary chunk (rows 24..31) runs first so
the AllReduce is in flight ~5us into the step; its consumers (next step's
boundary chunk + mini) run ~23us later, hiding the collective. On other
steps chunks run top-down (0,1,2,3,mini) so each chunk's rows are produced
a full step before the next step's consumer chunk reads them. The mini
group lives in its own 2-bank PSUM pool so its reuse cannot serialize the
main 6-bank chunk ring. Halo receive (sub + copies) and the BN affine run
on the otherwise-idle GpSimd queue; the halo send rows are packed
row-reversed (for the flipped peer) in two negative-stride DVE copies.

Queue discipline (the critical lesson of this kernel): any op that waits on
the collective blocks its whole in-order queue, so x loads ride the scalar
queue (prefetched two steps ahead, ring of 3) and never sit behind the y
stores on sync, whose BN producers can transiently stall behind the halo
receive on gpsimd. i and f gates share one 128-partition activation (c
lives on partitions 64:128 to match f's base partition).
"""
import os
import numpy as np

T, H2, W2, F, CIN = 16, 64, 64, 64, 32
WP, XW, NQ = 66, 130, 8
N_HALVES = 2
R = H2 // N_HALVES
E = 4                 # halo width / steps per exchange phase
EXT = E - 1           # max redundant rows per step
RX = R + EXT          # rows of conv output prepared per step
SLAB = 2 * RX + 1     # input rows needed per slab
HSLOT = R + 6         # h row-slot count (pad + R owned + ext/halo + pad)
NCHUNK = R // NQ
MM_DT = os.environ.get("CONV_LSTM_MM_DT", "bf16")  # bf16 | fp32 | fp32r

_CACHE = {}


def _storage_np_dtype():
    import ml_dtypes
    return ml_dtypes.bfloat16 if MM_DT == "bf16" else np.float32


def _prep_core_inputs(x, W, U, b, gamma, beta, moving_mean, moving_var,
                      bidx, half):
    sdt = _storage_np_dtype()
    flip = (half == 1)

    # x slab [T, CIN, SLAB, XW]; XLA SAME (stride2,k3,even) pads bottom/right
    # only: out row r reads input rows 2r..2r+2 (row/col 128 = zero pad).
    xs = np.zeros((T, CIN, SLAB, XW), np.float32)
    xc = np.ascontiguousarray(x[bidx].transpose(0, 3, 1, 2))  # (T,CIN,128,128)
    if not flip:
        xs[:, :, 0:SLAB, 0:128] = xc[:, :, 0:SLAB, :]
    else:
        # slab[s] = x_global[128 - s]; s=0 is the zero pad row
        xs[:, :, 1:SLAB, 0:128] = xc[:, :, 128 - SLAB + 1:][:, :, ::-1, :]

    Wk = W[::-1].copy() if flip else W
    Uk = U[::-1].copy() if flip else U

    w3 = np.zeros((96, 768), np.float32)
    ua = np.zeros((128, 768), np.float32)
    ub = np.zeros((128, 768), np.float32)
    for di in range(3):
        for m in range(2):
            g = di * 2 + m
            cols = slice(g * 128, (g + 1) * 128)
            mc = slice(m * 128, (m + 1) * 128)
            for j in range(3):
                w3[32*j:32*j+32, cols] = Wk[j, di, :, mc]
            ua[0:64, cols] = Uk[0, di, :, mc]
            ua[64:128, cols] = Uk[1, di, :, mc]
            ub[0:64, cols] = Uk[2, di, :, mc]

    eps = 1e-3
    scale = (gamma / np.sqrt(moving_var + eps)).astype(np.float32)
    beta2 = (beta - moving_mean * scale).astype(np.float32)
    vecs = np.zeros((128, 8), np.float32)
    vecs[:, 0] = 0.2 * b[0:128] + 0.5
    vecs[0:64, 1] = b[128:192]
    vecs[64:128, 2] = 0.2 * b[192:256] + 0.5
    vecs[0:64, 3] = scale
    vecs[0:64, 4] = beta2
    return {
        "xs": np.ascontiguousarray(xs.astype(sdt)),
        "w3": np.ascontiguousarray(w3.astype(sdt)),
        "ua": np.ascontiguousarray(ua.astype(sdt)),
        "ub": np.ascontiguousarray(ub.astype(sdt)),
        "vecs": vecs,
    }


def _patch_tile_drain():
    """This walrus build encodes at most ONE sync wait per CTRL instruction;
    split the Tile exit drain's waits across SP nops."""
    import bass_rust
    import concourse.tile as tile
    from concourse.vector_clock import ScopedClock
    if getattr(tile.TileContext, "_drain_patched", False):
        return

    def patched(self, tick_clock, wait_clock):
        drain_inst = self.nc.sync.drain()
        wait_clock.add_sem_waits(
            drain_inst.ins, ScopedClock({None: tick_clock.global_clock}))
        si = drain_inst.ins.sync_info
        waits = list(si.on_wait) if si is not None else []
        if len(waits) > 1:
            si.on_wait = waits[:1]
            for w in waits[1:]:
                nop = self.nc.sync.nop()
                nsi = nop.ins.sync_info
                if nsi is None:
                    nop.ins.sync_info = bass_rust.SyncInfo(
                        on_wait=[w], on_update=[])
                else:
                    nsi.on_wait = [w]
        self.nc.all_engine_barrier()
        assert self.sems is not None
        popped = self.nc._tile_sem_poison_stack.pop()
        assert popped is self._sem_poison
        self.nc.clear_and_free_semaphores(list(self.sems.allocated().values()))
        self.nc.all_engine_barrier()

    tile.TileContext._drain_and_barrier = patched
    tile.TileContext._drain_patched = True


def _split_multi_waits(nc, mybir):
    """This walrus build encodes at most one sync wait per instruction;
    move excess waits onto single-wait nops inserted just before."""
    ctr = 0
    for bb in nc.main_func.blocks:
        insts = bb.instructions
        out = []
        changed = False
        for inst in insts:
            si = inst.sync_info
            waits = list(si.on_wait) if si is not None else []
            if len(waits) > 1:
                changed = True
                for w in waits[:-1]:
                    ctr += 1
                    out.append(mybir.InstNoOp(
                        name=f"wsplit-{ctr}",
                        engine=inst.engine,
                        sync_info=mybir.SyncInfo(on_wait=[w], on_update=[]),
                        bass_nofuse=True))
                si.on_wait = [waits[-1]]
            out.append(inst)
        if changed:
            bb.instructions = out


def _build_nc():
    import concourse.bass as bass
    import concourse.mybir as mybir
    import concourse.tile as tile
    _patch_tile_drain()
    dt = mybir.dt
    sdt = dt.bfloat16 if MM_DT == "bf16" else dt.float32
    AF = mybir.ActivationFunctionType

    def mm_ap(ap):
        return ap.bitcast(dt.float32r) if MM_DT == "fp32r" else ap

    nc = bass.Bass()
    xs = nc.dram_tensor("xs", [T, CIN, SLAB, XW], sdt, kind="ExternalInput")
    w3 = nc.dram_tensor("w3", [96, 768], sdt, kind="ExternalInput")
    ua = nc.dram_tensor("ua", [128, 768], sdt, kind="ExternalInput")
    ub = nc.dram_tensor("ub", [128, 768], sdt, kind="ExternalInput")
    vecs = nc.dram_tensor("vecs", [128, 8], dt.float32, kind="ExternalInput")
    y = nc.dram_tensor("y", [T, F, R * W2], dt.float32, kind="ExternalOutput")

    groups = [[0, 1], [2, 3], [4, 5], [6, 7]]

    with tile.TileContext(nc) as tc:
        with (
            tc.tile_pool(name="const", bufs=1) as cpool,
            tc.tile_pool(name="state", bufs=1) as spool,
            tc.tile_pool(name="xp", bufs=3) as xpool,
            tc.tile_pool(name="ps", bufs=6, space="PSUM") as pspool,
            tc.tile_pool(name="psm", bufs=2, space="PSUM") as pmpool,
            tc.tile_pool(name="epi", bufs=3) as epool,
            tc.tile_pool(name="halo", bufs=2) as hpool,
            tc.tile_pool(name="dram", bufs=2, space="DRAM") as dpool,
        ):
            w3sb = cpool.tile([96, 768], sdt, tag="w3sb")
            uasb = cpool.tile([128, 768], sdt, tag="uasb")
            ubsb = cpool.tile([128, 768], sdt, tag="ubsb")
            vsb = cpool.tile([128, 8], dt.float32, tag="vsb")
            # keep the sync queue free for the x loads: weights ride the
            # scalar/gpsimd queues so the first matmul isn't serialized
            # behind them
            nc.scalar.dma_start(out=w3sb[:], in_=w3[:])
            nc.scalar.dma_start(out=uasb[:], in_=ua[:])
            nc.gpsimd.dma_start(out=ubsb[:], in_=ub[:])
            nc.gpsimd.dma_start(out=vsb[:], in_=vecs[:])

            # warm up the collective ring with a throwaway AllReduce of the
            # same shape/groups as the halo exchange: the first real CC
            # otherwise pays ~6us of one-time setup on the critical path.
            win_d = dpool.tile([64, 512], dt.float32, tag="win")
            wout_d = dpool.tile([64, 512], dt.float32, tag="wout")
            nc.gpsimd.collective_compute(
                "AllReduce", mybir.AluOpType.add, replica_groups=groups,
                ins=[win_d[:].opt()], outs=[wout_d[:].opt()])

            h2 = [spool.tile([128, HSLOT * WP], sdt, name=f"h2_{i}",
                             tag=f"h2_{i}")
                  for i in range(2)]
            # c lives on partitions 64:128 so the f-gate half of the merged
            # i/f activation (also at base 64) can multiply it directly
            c2 = spool.tile([128, (R + E) * W2], dt.float32, tag="c")
            nc.vector.memset(h2[0][:], 0.0)
            nc.vector.memset(h2[1][:], 0.0)
            nc.vector.memset(c2[:], 0.0)

            def conv_group(pss, x3r, hpr, q0, nrow):
                """Accumulate the 9-matmul conv group for rows q0..q0+nrow-1
                into PSUM tiles pss (one per gate-tile m)."""
                psrs = [ps[:].rearrange("p (a b) -> p a b", b=W2)[
                    :, 0:nrow, :] for ps in pss]
                for di in range(3):
                    d = di - 1
                    for m in range(2):
                        gcol = slice((di*2+m)*128, (di*2+m+1)*128)
                        nc.tensor.matmul(
                            psrs[m][:],
                            lhsT=mm_ap(w3sb[0:96, gcol]),
                            rhs=mm_ap(x3r[0:96, q0:q0+nrow,
                                          d+1:d+129:2]),
                            start=(di == 0), stop=False)
                for di in range(3):
                    d = di - 1
                    for m in range(2):
                        gcol = slice((di*2+m)*128, (di*2+m+1)*128)
                        nc.tensor.matmul(
                            psrs[m][:],
                            lhsT=mm_ap(uasb[0:128, gcol]),
                            rhs=mm_ap(hpr[0:128, q0:q0+nrow,
                                          1+d:65+d]),
                            start=False, stop=False)
                for di in range(3):
                    d = di - 1
                    for m in range(2):
                        gcol = slice((di*2+m)*128, (di*2+m+1)*128)
                        nc.tensor.matmul(
                            psrs[m][:],
                            lhsT=mm_ap(ubsb[0:128, gcol]),
                            rhs=mm_ap(hpr[0:128, q0+2:q0+nrow+2,
                                          1+d:65+d]),
                            start=False, stop=(di == 2))

            def epilogue(pss, hcr, q0, nrow, store_y, t):
                """Gates + LSTM cell update for rows q0..q0+nrow-1; writes h
                (both copies), c, and optionally y."""
                ps0, ps1 = pss
                psl = slice(0, nrow * W2)
                cs = slice(q0 * W2, (q0 + nrow) * W2)
                # i and f share the hard-sigmoid affine, so one 128-part
                # ACT covers both (i at 0:64, f at 64:128 like the PSUM
                # layout); c sits at base 64 so f*c has equal input bases.
                if_t = epool.tile([128, 512], dt.float32, tag="if")
                nc.scalar.activation(if_t[:, psl], ps0[0:128, psl], AF.Relu,
                                     bias=vsb[0:128, 0:1], scale=0.2)
                g_t = epool.tile([64, 512], dt.float32, tag="g")
                nc.scalar.activation(g_t[:, psl], ps1[0:64, psl], AF.Tanh,
                                     bias=vsb[0:64, 1:2], scale=1.0)
                o_t = epool.tile([64, 512], dt.float32, tag="o")
                nc.scalar.activation(o_t[:, psl], ps1[64:128, psl],
                                     AF.Relu,
                                     bias=vsb[64:128, 2:3], scale=0.2)
                # hard-sigmoid clip fused into the gate products:
                # t = (gate min 1.0) * other
                t1 = epool.tile([64, 512], dt.float32, tag="t1")
                nc.vector.scalar_tensor_tensor(
                    t1[:, psl], if_t[64:128, psl], 1.0, c2[64:128, cs],
                    mybir.AluOpType.min, mybir.AluOpType.mult)
                t2 = epool.tile([64, 512], dt.float32, tag="t2")
                nc.vector.scalar_tensor_tensor(
                    t2[:, psl], if_t[0:64, psl], 1.0, g_t[:, psl],
                    mybir.AluOpType.min, mybir.AluOpType.mult)
                nc.vector.tensor_add(c2[64:128, cs], t1[:, psl], t2[:, psl])
                tc_t = epool.tile([64, 512], dt.float32, tag="tc")
                nc.scalar.activation(tc_t[:, psl], c2[64:128, cs], AF.Tanh)
                hlo = hcr[0:64, q0+1:q0+nrow+1, 1:65]
                nc.vector.scalar_tensor_tensor(
                    hlo,
                    o_t[:, psl].rearrange("p (a b) -> p a b", b=W2), 1.0,
                    tc_t[:, psl].rearrange("p (a b) -> p a b", b=W2),
                    mybir.AluOpType.min, mybir.AluOpType.mult)
                nc.vector.tensor_copy(
                    out=hcr[64:128, q0:q0+nrow, 1:65], in_=hlo)
                if store_y:
                    yst = epool.tile([64, 512], dt.float32, tag="yst")
                    nc.gpsimd.tensor_scalar(
                        yst[:, psl].rearrange("p (a b) -> p a b", b=W2),
                        hlo,
                        vsb[0:64, 3:4], vsb[0:64, 4:5],
                        mybir.AluOpType.mult, mybir.AluOpType.add)
                    nc.sync.dma_start(out=y[t, :, q0*W2:(q0+nrow)*W2],
                                      in_=yst[:, psl])

            def load_x(tt, qs=None):
                """Issue the x row-parity loads for step tt. Default queue
                is scalar: the sync queue carries the y stores, which can
                transiently block behind the halo collective - x must never
                sit behind them."""
                qs = qs or (nc.scalar,) * 3
                x3t = xpool.tile([96, RX * XW], sdt, tag="x3",
                                 name=f"x3_{tt}")
                x3r = x3t[:].rearrange("p (q w) -> p q w", w=XW)
                qs[0].dma_start(out=x3r[0:32], in_=xs[tt, :, 0:2*RX-1:2, :])
                qs[1].dma_start(out=x3r[32:64], in_=xs[tt, :, 1:2*RX:2, :])
                qs[2].dma_start(out=x3r[64:96], in_=xs[tt, :, 2:2*RX+1:2, :])
                return x3r

            # t=0's load is the critical first input: a small leading
            # piece (rows q<9, enough for chunk 0) lands first across
            # three queues, then the rest follows
            x3t0 = xpool.tile([96, RX * XW], sdt, tag="x3", name="x3_0")
            x3r0 = x3t0[:].rearrange("p (q w) -> p q w", w=XW)
            nc.sync.dma_start(out=x3r0[0:32, 0:9], in_=xs[0, :, 0:17:2, :])
            nc.sync.dma_start(out=x3r0[32:64, 0:9], in_=xs[0, :, 1:18:2, :])
            nc.gpsimd.dma_start(out=x3r0[64:96, 0:9], in_=xs[0, :, 2:19:2, :])
            nc.sync.dma_start(out=x3r0[0:32, 9:RX],
                              in_=xs[0, :, 18:2*RX-1:2, :])
            nc.sync.dma_start(out=x3r0[32:64, 9:RX],
                              in_=xs[0, :, 19:2*RX:2, :])
            nc.gpsimd.dma_start(out=x3r0[64:96, 9:RX],
                                in_=xs[0, :, 20:2*RX+1:2, :])
            xbufs = {0: x3r0, 1: load_x(1)}
            for t in range(T):
                j = t % E          # phase position; e = EXT - j extra rows
                e = EXT - j
                hc = h2[t % 2]
                hp = h2[(t + 1) % 2]
                hcr = hc[:].rearrange("p (q w) -> p q w", w=WP)
                hpr = hp[:].rearrange("p (q w) -> p q w", w=WP)

                # prefetch two steps ahead (ring of 3) so trigger latency
                # never gates a matmul
                x3r = xbufs.pop(t)
                if t + 2 < T:
                    xbufs[t + 2] = load_x(t + 2)

                exchange = (j == E - 1 and t < T - 1)
                # On exchange steps a 4-row boundary group (rows 28..31 -
                # exactly the halo payload) runs first so the AllReduce is
                # in flight ~5us into the step; the rest of chunk 3, then
                # chunks 0-2, overlap the collective. Otherwise top-down
                # order keeps each chunk a full step ahead of its consumers.
                if exchange:
                    sched = [(3*NQ+4, 4, pmpool), (3*NQ, 4, pspool),
                             (0, NQ, pspool), (NQ, NQ, pspool),
                             (2*NQ, NQ, pspool)]
                else:
                    sched = [(0, NQ, pspool), (NQ, NQ, pspool),
                             (2*NQ, NQ, pspool), (3*NQ, NQ, pspool)]

                for q0, nrow, pool in sched:
                    width = 512 if pool is pspool else 4 * W2
                    pss = [pool.tile([128, width], dt.float32,
                                     name=f"ps_{t}_{q0}_{mi}", tag="ps"
                                     if pool is pspool else "psm")
                           for mi in range(2)]
                    conv_group(pss, x3r, hpr, q0, nrow)
                    epilogue(pss, hcr, q0, nrow, True, t)

                    if q0 == 3*NQ+4 and exchange:
                        # pack own boundary rows 28..31 of (h, c) row-
                        # reversed (the flipped peer consumes them in its
                        # own orientation), AllReduce with the paired core,
                        # subtract own contribution, scatter the peer rows
                        # into the halo slots.
                        bsend = hpool.tile([64, 512], dt.float32,
                                           tag="bsend")
                        # rows packed reversed (slot R-k for block k) via
                        # negative-stride APs: one copy for h, one for c
                        bsr = bsend[:].rearrange("p (a b) -> p a b", b=64)
                        nc.vector.tensor_copy(
                            out=bsr[:, 0:E, :],
                            in_=hcr[0:64, R:R-E:-1, 1:65])
                        c2r = c2[:].rearrange("p (a b) -> p a b", b=W2)
                        nc.vector.tensor_copy(
                            out=bsr[:, E:2*E, :],
                            in_=c2r[64:128, R-1:R-1-E:-1, :])
                        bin_d = dpool.tile([64, 512], dt.float32,
                                           tag="bin")
                        bout_d = dpool.tile([64, 512], dt.float32,
                                            tag="bout")
                        nc.gpsimd.dma_start(out=bin_d[:], in_=bsend[:])
                        nc.gpsimd.collective_compute(
                            "AllReduce", mybir.AluOpType.add,
                            replica_groups=groups,
                            ins=[bin_d[:].opt()], outs=[bout_d[:].opt()])
                        bsum = hpool.tile([64, 512], dt.float32,
                                          tag="bsum")
                        nc.gpsimd.dma_start(out=bsum[:], in_=bout_d[:])
                        # receive on the (otherwise idle) gpsimd queue so
                        # it runs the moment the collective lands. Each
                        # subtract writes its final destination directly
                        # (cast on output) - one hop per consumer, the
                        # A-copy first since next step's boundary chunk
                        # waits on it.
                        bsh = bsum[:, 0:256].rearrange(
                            "p (a b) -> p a b", b=64)
                        beh = bsend[:, 0:256].rearrange(
                            "p (a b) -> p a b", b=64)
                        nc.gpsimd.tensor_sub(
                            hcr[0:64, R+1:R+1+E, 1:65], bsh, beh)
                        nc.gpsimd.tensor_sub(
                            hcr[64:128, R:R+E, 1:65], bsh, beh)
                        nc.gpsimd.tensor_sub(
                            c2[64:128, R*W2:(R+E)*W2],
                            bsum[:, 256:512], bsend[:, 256:512])

                    if q0 == 0 and e == 1:
                        # pre-exchange step: the mini (row 32) runs right
                        # after chunk 0 so the next step's boundary group
                        # isn't gated on a step-tail DVE chain
                        mss = [pmpool.tile([128, 4 * W2], dt.float32,
                                           name=f"psm_{t}_{mi}", tag="psm")
                               for mi in range(2)]
                        conv_group(mss, x3r, hpr, R, e)
                        epilogue(mss, hcr, R, e, False, t)

                if e > 1:
                    # mini chunk: redundant neighbor rows 32..31+e keep the
                    # wedge alive between halo exchanges. No y store. (At
                    # e == 1 it ran after chunk 0 instead, above.)
                    q0 = R
                    pss = [pmpool.tile([128, 4 * W2], dt.float32,
                                       name=f"psm_{t}_{mi}", tag="psm")
                           for mi in range(2)]
                    conv_group(pss, x3r, hpr, q0, e)
                    epilogue(pss, hcr, q0, e, False, t)
    _split_multi_waits(nc, mybir)
    return nc


def _install_ntff_hook():
    """The image's antenv lacks axon_hooks; synthesize it and register the
    ctypes NTFF profile hook so trace=True works under axon."""
    import sys
    import types
    try:
        from antenv.axon_hooks import get_axon_ntff_profile_hook  # noqa
        return
    except ImportError:
        pass
    mod = types.ModuleType("antenv.axon_hooks")
    mod._hook = None

    def set_axon_ntff_profile_hook(h):
        mod._hook = h

    def get_axon_ntff_profile_hook():
        return mod._hook

    mod.set_axon_ntff_profile_hook = set_axon_ntff_profile_hook
    mod.get_axon_ntff_profile_hook = get_axon_ntff_profile_hook
    sys.modules["antenv.axon_hooks"] = mod
    import antenv
    antenv.axon_hooks = mod
    try:
        from trn_agent_boot.trn_boot import _ntff_profile_via_ctypes
        hook = _ntff_profile_via_ctypes("/opt/axon/libaxon_pjrt.so")
        if hook is not None:
            mod._hook = hook
    except Exception:
        pass


def _get_nc():
    key = (MM_DT,)
    if key not in _CACHE:
        _CACHE[key] = _build_nc()
    return _CACHE[key]


def kernel(x, W, U, b, gamma, beta, moving_mean, moving_var):
    from concourse.bass_utils import run_bass_kernel_spmd
    x = np.asarray(x, np.float32)
    W = np.asarray(W, np.float32)
    U = np.asarray(U, np.float32)
    b = np.asarray(b, np.float32)
    gamma = np.asarray(gamma, np.float32)
    beta = np.asarray(beta, np.float32)
    moving_mean = np.asarray(moving_mean, np.float32)
    moving_var = np.asarray(moving_var, np.float32)
    B = x.shape[0]

    in_maps = []
    for bidx in range(B):
        for half in range(N_HALVES):
            in_maps.append(_prep_core_inputs(
                x, W, U, b, gamma, beta, moving_mean, moving_var, bidx, half))

    nc = _get_nc()
    trace = os.environ.get("BASS_KERNEL_TRACE") == "1"
    if trace:
        _install_ntff_hook()
    res = run_bass_kernel_spmd(nc, in_maps, core_ids=list(range(8)),
                               trace=trace)
    kernel._last_result = res

    out = np.zeros((B, T, H2, W2, F), np.float32)
    ci = 0
    for bidx in range(B):
        for half in range(N_HALVES):
            yc = res.results[ci]["y"].reshape(T, F, R, W2)
            ci += 1
            yc = yc.transpose(0, 2, 3, 1)  # (T, R, W2, F)
            if half == 1:
                yc = yc[:, ::-1, :, :]
                out[bidx, :, 32:64] = yc
            else:
                out[bidx, :, 0:32] = yc
    return out
